# revision 1
# baseline (speedup 1.0000x reference)
"""Trainium2 Bass kernel for nn_MultiHeadedLinrec (linear attention).

Math (per batch element, reference semantics):
    q = elu(x_q @ Wq.T + bq)    [S, E] viewed as [S, H, d]
    k = elu(x_k @ Wk.T + bk)
    v = x_v @ Wv.T + bv
    k <- k / (||k||_seq * sqrt(S))     (per (h, d) column norm over S)
    q <- q / (||q||_d   * sqrt(d))     (per (s, h) row norm over d)
    scores_h = k_h^T @ v_h             [d, d]
    out = concat_h(q_h @ scores_h) @ Wo.T + bo

Kernel strategy (one NeuronCore per batch element, 8 cores data-parallel):
  The host pre-transposes activations to xT layout [E, S] and converts both
  activations and weights to bf16, so the device never runs PE transposes
  and never needs f32r rounding copies (bf16 moving operands run at
  1 cyc/row at any moving size; f32r needs N >= 256).  The walrus verifier
  rejects mixed f32r/bf16 matmul operands, so every matmul is either pure
  bf16 or pure f32r.

  Phase A (stream S in 128-row s-tiles, loaded as 512-col blocks of xT):
    project k/v into natural layout [s, e] with x-chunks stationary and
    bf16 weights moving, ELU(k), pack per-head [v | k] bf16 tiles, and
    accumulate per-head scoresT = v_h^T k_h plus the k-gram (for column
    norms) on the PE with bf16 moving at 1 cyc/row.  x blocks are
    prefetched one block ahead; WoT/WqT stream in during phase A so the
    later phases never wait on weight DMAs.
  Phase B: fold k-norm + scores + Wo into one fused weight
    W2[i, o] = (scores @ Wo.T)[i, o] / (knorm[i] * sqrt(S)),
    built as 8 block-diagonal 128x128 bf16 matmuls against WoT tiles.
    Issued between the first phase-C projection and the rest of the
    pipeline so the PE never idles across the phase boundary.
  Phase C (stream S in 512-col blocks): software-pipelined per block as
    projA (q projection in qT layout, ELU, sum-of-squares) -> normB
    (invq broadcast via PE, scale to bf16 qn) -> consume (out = qnT.T @ W2
    + bo in natural layout).  projA(b+1) is issued before normB(b) /
    consume(b) so the q-norm latency chain hides under matmul work.

This walrus build only supports ONE sync wait per instruction; Tile emits
multi-wait instructions, so we legalize the BIR JSON by hoisting extra waits
onto inserted NoOps (see _legalize_sync_json).
"""

import json

import numpy as np

import concourse.bass as bass
import concourse.bass_isa as bass_isa
import concourse.mybir as mybir
import concourse.tile as tile
from concourse.bass_utils import run_bass_kernel_spmd

dt = mybir.dt
AF = mybir.ActivationFunctionType
ALU = mybir.AluOpType

P = 128
E = 1024
H = 16
D = 64
N_CORES = 8
EC = E // P  # 8 chunks of 128 along the embedding dim
SBLK = 512  # s-block width for x loads / phase-C processing


# --------------------------------------------------------------------------
# BIR sync legalization: max one wait / one update per instruction.
# --------------------------------------------------------------------------
def _legalize_sync_json(bir_json: bytes) -> bytes:
    m = json.loads(bir_json)
    counter = [0]

    def fresh():
        counter[0] += 1
        return f"I-synclift-{counter[0]}"

    for f in m["functions"]:
        for blk in f["blocks"]:
            out = []
            for ins in blk["instructions"]:
                si = ins.get("sync_info")
                if not si:
                    out.append(ins)
                    continue
                waits = si.get("on_wait") or []
                updates = si.get("on_update") or []
                if len(waits) <= 1 and len(updates) <= 1:
                    out.append(ins)
                    continue
                eng = ins.get("engine")
                dbg = ins.get("debug")
                for w in waits[:-1]:
                    out.append(
                        {
                            "debug": dbg,
                            "engine": eng,
                            "ins": [],
                            "name": fresh(),
                            "opcode": "NoOp",
                            "outs": [],
                            "sync_info": {"on_update": [], "on_wait": [w]},
                        }
                    )
                si["on_wait"] = waits[-1:]
                post = [
                    {
                        "debug": dbg,
                        "engine": eng,
                        "ins": [],
                        "name": fresh(),
                        "opcode": "NoOp",
                        "outs": [],
                        "sync_info": {"on_update": [u], "on_wait": []},
                    }
                    for u in updates[1:]
                ]
                si["on_update"] = updates[:1]
                out.append(ins)
                out.extend(post)
            blk["instructions"] = out
    return json.dumps(m).encode()


def _patch_bass(nc):
    orig = nc.to_json_bytes

    def patched():
        return _legalize_sync_json(orig())

    nc.to_json_bytes = patched
    return nc


# --------------------------------------------------------------------------
# Kernel builder
# --------------------------------------------------------------------------
def build(S: int = 4096, with_bias: bool = True, cfg: dict | None = None):
    cfg = dict(cfg or {})
    ST = S // P  # number of 128-row s-tiles
    NBLK = S // SBLK  # number of 512-col s-blocks
    JB = SBLK // P  # s-tiles per block (4)

    nc = bass.Bass(trn_type="TRN2", target_bir_lowering=False, debug=False)

    f32 = dt.float32
    f32r = dt.float32r
    bf16 = dt.bfloat16

    xqT = nc.dram_tensor("xqT", [E, S], bf16, kind="ExternalInput").ap()
    xkT = nc.dram_tensor("xkT", [E, S], bf16, kind="ExternalInput").ap()
    xvT = nc.dram_tensor("xvT", [E, S], bf16, kind="ExternalInput").ap()
    WqTd = nc.dram_tensor("WqT", [E, E], bf16, kind="ExternalInput").ap()
    WkTd = nc.dram_tensor("WkT", [E, E], bf16, kind="ExternalInput").ap()
    WvTd = nc.dram_tensor("WvT", [E, E], bf16, kind="ExternalInput").ap()
    WoTd = nc.dram_tensor("WoT", [E, E], bf16, kind="ExternalInput").ap()
    bonesd = nc.dram_tensor("bones", [P, EC * H], bf16, kind="ExternalInput").ap()
    bpickd = nc.dram_tensor("bpick", [H, EC * P], bf16, kind="ExternalInput").ap()
    bq = nc.dram_tensor("bq", [1, E], f32, kind="ExternalInput").ap()
    bk = nc.dram_tensor("bk", [1, E], f32, kind="ExternalInput").ap()
    bv = nc.dram_tensor("bv", [1, E], f32, kind="ExternalInput").ap()
    bo = nc.dram_tensor("bo", [1, E], f32, kind="ExternalInput").ap()
    out = nc.dram_tensor("out", [S, E], f32, kind="ExternalOutput").ap()
    taps = {}
    if cfg.get("taps"):
        taps = {
            "tap_kv0": nc.dram_tensor("tap_kv0", [P, 2 * E], bf16, kind="ExternalOutput").ap(),
            "tap_gram": nc.dram_tensor("tap_gram", [D, H * D], f32, kind="ExternalOutput").ap(),
            "tap_kcol": nc.dram_tensor("tap_kcol", [P, EC], f32, kind="ExternalOutput").ap(),
            "tap_invk": nc.dram_tensor("tap_invk", [P, EC], f32, kind="ExternalOutput").ap(),
            "tap_bd0": nc.dram_tensor("tap_bd0", [P, P], f32, kind="ExternalOutput").ap(),
            "tap_w20": nc.dram_tensor("tap_w20", [P, E], bf16, kind="ExternalOutput").ap(),
            "tap_qss": nc.dram_tensor("tap_qss", [H, SBLK], f32, kind="ExternalOutput").ap(),
            "tap_qt0": nc.dram_tensor("tap_qt0", [P, SBLK], bf16, kind="ExternalOutput").ap(),
            "tap_qn0": nc.dram_tensor("tap_qn0", [P, SBLK], bf16, kind="ExternalOutput").ap(),
        }

    with tile.TileContext(nc) as tc:
        with (
            tc.tile_pool(name="consts", bufs=1) as consts,
            tc.tile_pool(name="small", bufs=1) as small,
            tc.tile_pool(name="drpool", bufs=1, space="DRAM") as drpool,
            tc.tile_pool(name="wts_kv", bufs=1) as wts_kv,
            tc.tile_pool(name="wts_o", bufs=1) as wts_o,
            tc.tile_pool(name="wts_q", bufs=1) as wts_q,
            tc.tile_pool(name="w2pool", bufs=1) as w2pool,
            tc.tile_pool(name="c_in", bufs=2) as c_in,
        ):
            # ---------------- constants ----------------
            # zero128 memset on the idle gpsimd queue: the PE's zero-init
            # matmuls are its first work and DVE has preamble backlog
            zero128 = consts.tile([P, P], bf16, name="zero128")
            nc.gpsimd.memset(zero128[:], 0.0)
            ones_1x128 = None
            if with_bias:
                ones_st = consts.tile([1, P], f32, name="ones_st")
                nc.vector.memset(ones_st[:], 1.0)
                ones_1x128 = consts.tile([1, P], f32r, name="ones_1x128")
                nc.vector.tensor_copy(ones_1x128[:], ones_st[:])

            # block-ones / block-pick patterns arrive as bf16 host consts
            # (DMAs issued inside phase A, after the startup-critical loads)
            bones_sb = consts.tile([P, EC * H], bf16, name="bones_sb")
            bpick_sb = consts.tile([H, EC * P], bf16, name="bpick_sb")
            blockones = [bones_sb[:, c * H : (c + 1) * H] for c in range(EC)]
            blockpick = [bpick_sb[:, c * P : (c + 1) * P] for c in range(EC)]

            # ---------------- biases ----------------
            rows_scope = tc.tile_pool(name="rows", bufs=1)
            rows_pool = rows_scope.__enter__()
            bk_row = bv_row = bo_bcast = bq_col = None
            if with_bias:
                with tc.tile_pool(name="brow_stage", bufs=2) as stage_pool:
                    def load_row_r(name, src):
                        stage = stage_pool.tile([1, E], f32, name="brow_stage")
                        nc.sync.dma_start(stage[:], src)
                        row = rows_pool.tile([1, E], f32r, name=f"{name}_r")
                        nc.vector.tensor_copy(row[:], stage[:])
                        return row

                    bk_row = load_row_r("bk", bk)
                    bv_row = load_row_r("bv", bv)
                    bo_row = load_row_r("bo", bo)

                bq_col = small.tile([P, EC], f32, name="bq_col")
                nc.sync.dma_start(bq_col[:], bq.rearrange("1 (t p) -> p t", p=P))

                with tc.tile_pool(name="bias_ps", bufs=2, space="PSUM") as bias_ps:
                    def bcast_row(row_r, name):
                        full = small.tile([P, E], f32, name=f"{name}_bcast")
                        for h in range(2):
                            pt = bias_ps.tile([P, 512], f32, name="bias_ps")
                            nc.tensor.matmul(
                                pt[:],
                                ones_1x128[:],
                                row_r[:, h * 512 : (h + 1) * 512],
                                start=True,
                                stop=True,
                            )
                            nc.vector.tensor_copy(
                                full[:, h * 512 : (h + 1) * 512], pt[:]
                            )
                        return full

                    bo_bcast = bcast_row(bo_row, "bo")

            # bf16 weights arrive pre-transposed in DRAM; straight DMA
            def load_wt(WTd, name, dst_pool):
                tiles = []
                for c in range(EC):
                    t = dst_pool.tile([P, E], bf16, name=f"{name}T_{c}")
                    nc.sync.dma_start(t[:], WTd[c * P : (c + 1) * P, :])
                    tiles.append(t)
                return tiles

            bd_st = [
                small.tile([P, P], f32, name=f"bd_st_{pr}") for pr in range(8)
            ]

            WoT = WqT = None

            def load_xq(blk_i):
                xb = c_in.tile([P, EC * SBLK], bf16, name="xq_blk")
                nc.sync.dma_start(
                    xb[:].rearrange("p (c s) -> p c s", c=EC),
                    xqT[:, blk_i * SBLK : (blk_i + 1) * SBLK].rearrange(
                        "(c p) s -> p c s", p=P
                    ),
                )
                return xb

            # ================= PHASE A ====================================
            with (
                tc.tile_pool(name="a_in", bufs=cfg.get("a_in", 2)) as a_in,
                tc.tile_pool(name="a_act", bufs=cfg.get("a_act", 2)) as a_act,
                tc.tile_pool(name="a_kv", bufs=cfg.get("a_kv", 4)) as a_kv,
                tc.tile_pool(name="a_pj_ps", bufs=cfg.get("a_pj_ps", 5), space="PSUM") as a_pj_ps,
                tc.tile_pool(name="a_sc_ps", bufs=1, space="PSUM") as a_sc_ps,
            ):
                def load_xblk(x_src, blk_i, name):
                    """One 512-col block of xT: [128, (c s)] bf16."""
                    xb = a_in.tile([P, EC * SBLK], bf16, name=f"{name}_blk")
                    nc.sync.dma_start(
                        xb[:].rearrange("p (c s) -> p c s", c=EC),
                        x_src[:, blk_i * SBLK : (blk_i + 1) * SBLK].rearrange(
                            "(c p) s -> p c s", p=P
                        ),
                    )
                    return xb

                # first x block before the weights, so the DMA queue hands
                # the PE its first operands as early as possible
                def load_wt_halves(WTd, name, dst_pool):
                    tiles = [
                        dst_pool.tile([P, E], bf16, name=f"{name}T_{c}")
                        for c in range(EC)
                    ]
                    wengs = (
                        (nc.scalar, nc.gpsimd)
                        if cfg.get("mq_dma", False)
                        else (nc.sync, nc.sync)
                    )
                    for h in range(2):
                        for c in range(EC):
                            wengs[h].dma_start(
                                tiles[c][:, h * 512 : (h + 1) * 512],
                                WTd[c * P : (c + 1) * P,
                                    h * 512 : (h + 1) * 512],
                            )
                    return tiles

                def xk0_half(xk_blk, sh):
                    nc.sync.dma_start(
                        xk_blk[:]
                        .rearrange("p (c s) -> p c s", c=EC)[
                            :, :, sh * 256 : (sh + 1) * 256
                        ],
                        xkT[:, sh * 256 : sh * 256 + 256].rearrange(
                            "(c p) s -> p c s", p=P
                        ),
                    )

                if cfg.get("xk0_split", True):
                    # interleave: xk halves on the SP queue while the Wk
                    # chunk dispatches run in parallel on the idle vector /
                    # scalar queues — only the transfers serialize
                    weng = (
                        (nc.scalar, nc.gpsimd)
                        if cfg.get("mq_dma", False)
                        else (nc.sync, nc.sync)
                    )
                    xk_blk = a_in.tile([P, EC * SBLK], bf16, name="xk_blk")
                    xk0_half(xk_blk, 0)
                    WkT = [
                        wts_kv.tile([P, E], bf16, name=f"WkT_{c}")
                        for c in range(EC)
                    ]
                    for c in range(4):
                        weng[0].dma_start(
                            WkT[c][:, 0:512], WkTd[c * P : (c + 1) * P, 0:512]
                        )
                    xk0_half(xk_blk, 1)
                    for c in range(4, EC):
                        weng[0].dma_start(
                            WkT[c][:, 0:512], WkTd[c * P : (c + 1) * P, 0:512]
                        )
                    for c in range(EC):
                        weng[1].dma_start(
                            WkT[c][:, 512:1024],
                            WkTd[c * P : (c + 1) * P, 512:1024],
                        )
                else:
                    xk_blk = load_xblk(xkT, 0, "xk")
                    WkT = load_wt_halves(WkTd, "Wk", wts_kv)
                xv_blk = load_xblk(xvT, 0, "xv")
                WvT = load_wt_halves(WvTd, "Wv", wts_kv)

                scores_ps = a_sc_ps.tile([P, H * D], f32, name="scores_ps")
                zero_pending = list(range(8))

                def emit_zeros(n):
                    for _ in range(n):
                        if not zero_pending:
                            return
                        qtr = zero_pending.pop(0)
                        nc.tensor.matmul(
                            scores_ps[:, qtr * P : (qtr + 1) * P],
                            zero128[:],
                            zero128[:],
                            start=True,
                            stop=True,
                            skip_group_check=True,
                        )

                emit_zeros(4 if cfg.get("defer_zeros", False) else 8)

                def project_nat(xb, t, WT, brow, ps_pool):
                    """k/v projection for s-subtile t: x chunks stationary,
                    bf16 weights moving. Returns psum halves [128, 512]."""
                    halves = []
                    for h in range(2):
                        pj = ps_pool.tile([P, 512], f32, name="pj")
                        for c in range(EC):
                            nc.tensor.matmul(
                                pj[:],
                                xb[:, c * SBLK + t * P : c * SBLK + (t + 1) * P],
                                WT[c][:, h * 512 : (h + 1) * 512],
                                start=(c == 0),
                                stop=(brow is None and c == EC - 1),
                            )
                        if brow is not None:
                            nc.tensor.matmul(
                                pj[:],
                                ones_1x128[:],
                                brow[:, h * 512 : (h + 1) * 512],
                                start=False,
                                stop=True,
                            )
                        halves.append(pj)
                    return halves

                def do_kproj(it):
                    """k projection + ELU into a fresh kv tile for s-tile it."""
                    t = it % JB
                    kv_sb = a_kv.tile([P, 2 * E], bf16, name="kv_sb")
                    kv4 = kv_sb[:].rearrange("p (hh two) -> p hh two", two=2 * D)
                    kp = project_nat(xk_blk, t, WkT, bk_row, a_pj_ps)
                    for h in range(2):
                        r_sb = a_act.tile([P, 512], f32, name="kr_sb")
                        t_sb = a_act.tile([P, 512], f32, name="kt_sb")
                        e_sb = a_act.tile([P, 512], f32, name="ke_sb")
                        nc.scalar.activation(r_sb[:], kp[h][:], AF.Relu)
                        # elu(x) = relu(x) + min(exp(x), 1) - 1
                        nc.scalar.activation(e_sb[:], kp[h][:], AF.Exp)
                        nc.vector.tensor_scalar(
                            t_sb[:], e_sb[:], 1.0, -1.0, ALU.min, ALU.add
                        )
                        (nc.gpsimd if cfg.get("a_tt_pool", False) else nc.vector).tensor_tensor(
                            kv4[:, 8 * h : 8 * (h + 1), D : 2 * D],
                            t_sb[:].rearrange("p (hh d) -> p hh d", d=D),
                            r_sb[:].rearrange("p (hh d) -> p hh d", d=D),
                            ALU.add,
                        )
                    return kv_sb, kv4

                def do_vproj(it, kv4):
                    t = it % JB
                    vp = project_nat(xv_blk, t, WvT, bv_row, a_pj_ps)
                    for h in range(2):
                        nc.scalar.copy(
                            kv4[:, 8 * h : 8 * (h + 1), 0:D],
                            vp[h][:].rearrange("p (hh d) -> p hh d", d=D),
                        )

                def do_scores(it, kv_sb):
                    for hh in range(H):
                        nc.tensor.matmul(
                            scores_ps[:, hh * D : (hh + 1) * D],
                            kv_sb[:, 2 * D * hh : 2 * D * (hh + 1)],
                            kv_sb[:, 2 * D * hh + D : 2 * D * (hh + 1)],
                            start=False,
                            stop=(it == ST - 1 and hh == H - 1),
                            skip_group_check=True,
                        )

                def proj_half(xb, t, WT, brow, h, fill=False):
                    pj = a_pj_ps.tile([P, 512], f32, name="pj")
                    for c in range(EC):
                        nc.tensor.matmul(
                            pj[:],
                            xb[:, c * SBLK + t * P : c * SBLK + (t + 1) * P],
                            WT[c][:, h * 512 : (h + 1) * 512],
                            start=(c == 0),
                            stop=(brow is None and c == EC - 1),
                        )
                        if fill and c % 2 == 1:
                            emit_zeros(1)
                    if brow is not None:
                        nc.tensor.matmul(
                            pj[:], ones_1x128[:],
                            brow[:, h * 512 : (h + 1) * 512],
                            start=False, stop=True,
                        )
                    return pj

                def k_elu_half(kv4, h, kp_h):
                    edt = bf16 if cfg.get("elu_bf16", False) else f32
                    r_sb = a_act.tile([P, 512], edt, name="kr_sb")
                    t_sb = a_act.tile([P, 512], edt, name="kt_sb")
                    e_sb = a_act.tile([P, 512], edt, name="ke_sb")
                    nc.scalar.activation(r_sb[:], kp_h[:], AF.Relu)
                    # elu(x) = relu(x) + min(exp(x), 1) - 1
                    nc.scalar.activation(e_sb[:], kp_h[:], AF.Exp)
                    nc.vector.tensor_scalar(
                        t_sb[:], e_sb[:], 1.0, -1.0, ALU.min, ALU.add
                    )
                    nc.vector.tensor_tensor(
                        kv4[:, 8 * h : 8 * (h + 1), D : 2 * D],
                        t_sb[:].rearrange("p (hh d) -> p hh d", d=D),
                        r_sb[:].rearrange("p (hh d) -> p hh d", d=D),
                        ALU.add,
                    )

                # block 0 runs half-major (k-h0 for all subtiles, then k-h1,
                # then v halves, then scores) so the PE starts as soon as the
                # first Wk h0 chunks land and never outruns the DMA stream
                kv_b0 = []
                for t in range(JB):
                    kv_sb = a_kv.tile([P, 2 * E], bf16, name="kv_sb")
                    kv4 = kv_sb[:].rearrange(
                        "p (hh two) -> p hh two", two=2 * D
                    )
                    kv_b0.append((kv_sb, kv4))
                for h in range(2):
                    for t in range(JB):
                        fill = cfg.get("zfill", False) and h == 0 and t < 2
                        k_elu_half(kv_b0[t][1], h,
                                   proj_half(xk_blk, t, WkT, bk_row, h,
                                             fill=fill))
                emit_zeros(8)
                for h in range(2):
                    for t in range(JB):
                        vp_h = proj_half(xv_blk, t, WvT, bv_row, h)
                        nc.scalar.copy(
                            kv_b0[t][1][:, 8 * h : 8 * (h + 1), 0:D],
                            vp_h[:].rearrange("p (hh d) -> p hh d", d=D),
                        )
                for t in range(JB):
                    do_scores(t, kv_b0[t][0])

                xk_nxt = load_xblk(xkT, 1, "xk")
                xv_nxt = load_xblk(xvT, 1, "xv")

                for it in range(JB, ST):
                    blk_i, t = divmod(it, JB)
                    if t == 0:
                        xk_blk, xv_blk = xk_nxt, xv_nxt
                    if t == 0 and blk_i + 1 < NBLK:
                        xk_nxt = load_xblk(xkT, blk_i + 1, "xk")
                        xv_nxt = load_xblk(xvT, blk_i + 1, "xv")
                    if it == 4:
                        nc.sync.dma_start(bones_sb[:], bonesd)
                        nc.sync.dma_start(bpick_sb[:], bpickd)
                        WoT = load_wt(WoTd, "Wo", wts_o)
                    if it == 5:
                        WqT = load_wt(WqTd, "Wq", wts_q)
                    if it == 26:
                        xq0_blk = load_xq(0)

                    kv_sb, kv4 = do_kproj(it)
                    do_vproj(it, kv4)
                    do_scores(it, kv_sb)

                for s_t in bd_st:
                    nc.vector.memset(s_t[:], 0.0)

                # -- extract scoresT + ksumsq while phase-A psum still alive
                # Gram rows (64:128) hold k^T k per head; diagonal = ksumsq
                gram_sb = small.tile([D, H * D], f32, name="gram_sb")
                nc.vector.tensor_copy(gram_sb[:], scores_ps[D:P, :])
                gram_dram = drpool.tile([1, D * H * D], f32, name="gram_dram")
                nc.sync.dma_start(
                    gram_dram[:].rearrange("1 (d c) -> d c", d=D), gram_sb[:]
                )
                # diag idx for (hh, d) = d*(H*D) + hh*D + d = d*(H*D+1) + D*hh
                kcol = small.tile([P, EC], f32, name="kcol")
                gd = gram_dram[:].tensor
                for h2 in range(2):
                    src_ap = bass.AP(
                        gd, h2 * D, [[H * D + 1, D], [2 * D, EC]]
                    )
                    nc.sync.dma_start(kcol[h2 * D : (h2 + 1) * D, :], src_ap)
                if taps:
                    nc.sync.dma_start(taps["tap_gram"], gram_sb[:])
                    nc.sync.dma_start(taps["tap_kcol"], kcol[:])
                knorm = small.tile([P, EC], f32, name="knorm")
                nc.scalar.activation(knorm[:], kcol[:], AF.Sqrt, scale=float(S))
                invk = small.tile([P, EC], f32, name="invk")
                nc.vector.reciprocal(invk[:], knorm[:])
                if taps:
                    nc.sync.dma_start(taps["tap_invk"], invk[:])

                bd = []
                for pr in range(8):
                    h0, h1 = 2 * pr, 2 * pr + 1
                    nc.scalar.copy(
                        bd_st[pr][0:D, 0:D], scores_ps[0:D, h0 * D : (h0 + 1) * D]
                    )
                    odd_stage = small.tile([D, D], f32, name="odd_stage")
                    nc.scalar.copy(
                        odd_stage[:], scores_ps[0:D, h1 * D : (h1 + 1) * D]
                    )
                    nc.sync.dma_start(bd_st[pr][D:P, D:P], odd_stage[:])
                    bd_t = small.tile([P, P], bf16, name=f"bd_{pr}")
                    nc.vector.tensor_copy(bd_t[:], bd_st[pr][:])
                    bd.append(bd_t)
                if taps:
                    nc.sync.dma_start(taps["tap_bd0"], bd_st[0][:])

            rows_scope.__exit__(None, None, None)

            # ============ PHASE B + C: software-pipelined q pass ==========
            W2 = [w2pool.tile([P, E], bf16, name=f"W2_{c}") for c in range(EC)]
            with (
                tc.tile_pool(name="c_qt", bufs=cfg.get("c_qt", 2)) as c_qt,
                tc.tile_pool(name="c_qn", bufs=cfg.get("c_qn", 2)) as c_qn,
                tc.tile_pool(name="c_tmp", bufs=cfg.get("c_tmp", 2)) as c_tmp,
                tc.tile_pool(name="c_out", bufs=cfg.get("c_out", 2)) as c_out,
                tc.tile_pool(name="c_fin_ps", bufs=cfg.get("c_fin_ps", 2), space="PSUM") as c_fin_ps,
                tc.tile_pool(name="c_pj_ps", bufs=cfg.get("c_pj_ps", 3), space="PSUM") as c_pj_ps,
                tc.tile_pool(name="c_ss_ps", bufs=cfg.get("c_ss_ps", 2), space="PSUM") as c_ss_ps,
                tc.tile_pool(name="c_pj0_ps", bufs=1, space="PSUM") as c_pj0_ps,
                tc.tile_pool(name="c_qb", bufs=cfg.get("c_qb", 2)) as c_qb,
                tc.tile_pool(name="c_dr", bufs=2, space="DRAM") as c_dr,
            ):
                xq_pre = {}

                def projA(blk_i, xq_blk=None, qss_late=False):
                    """q projection (qT layout) + ELU + row sum-of-squares."""
                    if xq_blk is None:
                        xq_blk = xq_pre.pop(blk_i, None) or load_xq(blk_i)
                    if blk_i + 1 < NBLK:
                        xq_pre[blk_i + 1] = load_xq(blk_i + 1)
                    qss_ps = c_ss_ps.tile([H, SBLK], f32, name="qss_ps")
                    qt_tiles = []
                    q2_tiles = []
                    for ot in range(EC):
                        if blk_i == 0 and ot == 0 and cfg.get("pj0_bank", True):
                            pj = c_pj0_ps.tile([P, SBLK], f32, name="pj0")
                        else:
                            pj = c_pj_ps.tile([P, SBLK], f32, name="q_pj")
                        for c in range(EC):
                            nc.tensor.matmul(
                                pj[:],
                                WqT[c][:, ot * P : (ot + 1) * P],
                                xq_blk[:, c * SBLK : (c + 1) * SBLK],
                                start=(c == 0),
                                stop=(c == EC - 1),
                            )
                        edt = bf16 if cfg.get("elu_bf16", False) else f32
                        r_sb = c_tmp.tile([P, SBLK], edt, name="qr_sb")
                        t_sb = c_tmp.tile([P, SBLK], edt, name="qt_sb")
                        e_sb = c_tmp.tile([P, SBLK], edt, name="qe_sb")
                        qt_ = c_qt.tile([P, SBLK], bf16, name=f"qt_{ot}")
                        qbias = bq_col[:, ot : ot + 1] if with_bias else 0.0
                        nc.scalar.activation(r_sb[:], pj[:], AF.Relu, bias=qbias)
                        # elu(x) = relu(x) + min(exp(x), 1) - 1
                        nc.scalar.activation(e_sb[:], pj[:], AF.Exp, bias=qbias)
                        nc.vector.tensor_scalar(
                            t_sb[:], e_sb[:], 1.0, -1.0, ALU.min, ALU.add
                        )
                        (nc.gpsimd if cfg.get("c_tt_pool", False) else nc.vector).tensor_tensor(
                            qt_[:], t_sb[:], r_sb[:], ALU.add
                        )
                        qt_tiles.append(qt_)
                        q2 = c_qt.tile([P, SBLK], bf16, name=f"q2_{ot}")
                        nc.vector.tensor_tensor(q2[:], qt_[:], qt_[:], ALU.mult)
                        q2_tiles.append(q2)
                        if not qss_late:
                            nc.tensor.matmul(
                                qss_ps[:], blockones[ot], q2[:],
                                start=(ot == 0), stop=(ot == EC - 1),
                            )
                    if qss_late:
                        def qss_mm():
                            for ot in range(EC):
                                nc.tensor.matmul(
                                    qss_ps[:], blockones[ot], q2_tiles[ot][:],
                                    start=(ot == 0), stop=(ot == EC - 1),
                                )
                        return qt_tiles, qss_ps, qss_mm
                    return _finish_ss(qt_tiles, qss_ps)

                def _finish_ss(qt_tiles, qss_ps):
                    qss_sb = c_tmp.tile([H, SBLK], f32, name="qss_sb")
                    nc.scalar.activation(qss_sb[:], qss_ps[:], AF.Sqrt,
                                         scale=float(D))
                    if taps and blk_i == 0:
                        nc.sync.dma_start(taps["tap_qss"], qss_sb[:])
                        nc.sync.dma_start(taps["tap_qt0"], qt_tiles[0][:])
                    invq_b = c_tmp.tile([H, SBLK], bf16, name="invq_b")
                    with nc.allow_low_precision(
                        reason="invq rounds to bf16 exactly as the prior "
                               "f32-reciprocal + bf16-copy pair did"
                    ):
                        nc.vector.reciprocal(invq_b[:], qss_sb[:])
                    invq_dr = c_dr.tile([1, H * SBLK], bf16, name="invq_dr")
                    nc.sync.dma_start(
                        invq_dr[:].rearrange("1 (h s) -> h s", h=H), invq_b[:]
                    )
                    return qt_tiles, invq_dr, invq_b

                def projA_pool(blk_i, xq_blk=None):
                    """q projection + ELU + per-ot partition-reduce norms on
                    the idle gpsimd engine; returns finished qn tiles (no
                    block-level norm barrier, no qss matmuls, no qb DMAs)."""
                    if xq_blk is None:
                        xq_blk = xq_pre.pop(blk_i, None) or load_xq(blk_i)
                    if blk_i + 1 < NBLK:
                        xq_pre[blk_i + 1] = load_xq(blk_i + 1)
                    qn_tiles = []
                    for ot in range(EC):
                        pj = c_pj_ps.tile([P, SBLK], f32, name="q_pj")
                        for c in range(EC):
                            nc.tensor.matmul(
                                pj[:],
                                WqT[c][:, ot * P : (ot + 1) * P],
                                xq_blk[:, c * SBLK : (c + 1) * SBLK],
                                start=(c == 0),
                                stop=(c == EC - 1),
                            )
                        r_sb = c_tmp.tile([P, SBLK], f32, name="qr_sb")
                        t_sb = c_tmp.tile([P, SBLK], f32, name="qt_sb")
                        e_sb = c_tmp.tile([P, SBLK], f32, name="qe_sb")
                        qt_ = c_qt.tile([P, SBLK], bf16, name=f"qt_{ot}")
                        qbias = bq_col[:, ot : ot + 1] if with_bias else 0.0
                        nc.scalar.activation(r_sb[:], pj[:], AF.Relu, bias=qbias)
                        # elu(x) = relu(x) + min(exp(x), 1) - 1
                        nc.scalar.activation(e_sb[:], pj[:], AF.Exp, bias=qbias)
                        nc.vector.tensor_scalar(
                            t_sb[:], e_sb[:], 1.0, -1.0, ALU.min, ALU.add
                        )
                        nc.vector.tensor_tensor(
                            qt_[:], t_sb[:], r_sb[:], ALU.add
                        )
                        q2 = c_qt.tile([P, SBLK], bf16, name=f"q2_{ot}")
                        nc.vector.tensor_tensor(q2[:], qt_[:], qt_[:], ALU.mult)
                        qsum = c_qb.tile([P, SBLK], f32, name="qsum")
                        with nc.allow_low_precision(
                            reason="bf16 q^2 partition-reduce upcasts to f32"
                        ):
                            for hf in range(2):
                                nc.gpsimd.partition_all_reduce(
                                    qsum[hf * D : (hf + 1) * D, :],
                                    q2[hf * D : (hf + 1) * D, :],
                                    channels=D,
                                    reduce_op=bass_isa.ReduceOp.add,
                                )
                        qs_sb = c_tmp.tile([P, SBLK], f32, name="qs_sqrt")
                        nc.scalar.activation(qs_sb[:], qsum[:], AF.Sqrt,
                                             scale=float(D))
                        invq_o = c_qb.tile([P, SBLK], bf16, name="invq_o")
                        with nc.allow_low_precision(
                            reason="invq rounds to bf16 like the qb path"
                        ):
                            nc.vector.reciprocal(invq_o[:], qs_sb[:])
                        qn = c_qn.tile([P, SBLK], bf16, name=f"qn_{ot}")
                        nc.vector.tensor_tensor(
                            qn[:], qt_[:], invq_o[:], ALU.mult
                        )
                        qn_tiles.append(qn)
                    return qn_tiles

                def normB(state, on_pe=False, split4=False):
                    """invq broadcast (fused stride-0 DMAs; the tail block
                    splits by chunk-quads so fin consumes qn chunks as their
                    broadcast lands) + scale to bf16 qn tiles."""
                    qt_tiles, invq_dr, invq_b = state
                    dr = invq_dr[:].tensor
                    qn_tiles = []
                    qb_all = None
                    if not on_pe:
                        qb_all = c_qb.tile([P, EC * SBLK], bf16, name="qb_sb")
                        og = split4 if split4 else EC
                        for g in range(EC // og):
                            for hf in range(2):
                                nc.sync.dma_start(
                                    qb_all[
                                        hf * D : (hf + 1) * D,
                                        g * og * SBLK : (g + 1) * og * SBLK,
                                    ].rearrange("p (c s) -> p c s", c=og),
                                    bass.AP(
                                        dr, (2 * g * og + hf) * SBLK,
                                        [[0, D], [2 * SBLK, og], [1, SBLK]],
                                    ),
                                )
                    for ot in range(EC):
                        if on_pe:
                            qb = c_pj_ps.tile([P, SBLK], f32, name="q_pj")
                            nc.tensor.matmul(
                                qb[:], blockpick[ot], invq_b[:],
                                start=True, stop=True,
                            )
                            qb = qb[:]
                        else:
                            qb = qb_all[:, ot * SBLK : (ot + 1) * SBLK]
                        qn = c_qn.tile([P, SBLK], bf16, name=f"qn_{ot}")
                        nc.vector.tensor_tensor(
                            qn[:], qt_tiles[ot][:], qb, ALU.mult
                        )
                        qn_tiles.append(qn)
                    return qn_tiles

                def consume(blk_i, qn_tiles):
                    """out = qnT.T @ W2 (+ bo) in natural layout."""
                    s0 = blk_i * SBLK
                    if blk_i == NBLK - 1 and cfg.get("fine_tail", True):
                        # tail block: last s-tile streams out per half so
                        # the final copy+DMA chain after the last fin is short
                        for j in range(JB):
                            o_sb = c_out.tile([P, E], f32, name="o_sb")
                            for h in range(2):
                                fin = c_fin_ps.tile([P, 512], f32, name="fin_ps")
                                for c in range(EC):
                                    nc.tensor.matmul(
                                        fin[:],
                                        qn_tiles[c][:, j * P : (j + 1) * P],
                                        W2[c][:, h * 512 : (h + 1) * 512],
                                        start=(c == 0),
                                        stop=(c == EC - 1),
                                    )
                                sl = slice(h * 512, (h + 1) * 512)
                                if with_bias:
                                    nc.vector.scalar_tensor_tensor(
                                        o_sb[:, sl], fin[:], 0.0,
                                        bo_bcast[:, sl],
                                        ALU.add, ALU.add,
                                    )
                                elif h == 0:
                                    nc.vector.tensor_copy(o_sb[:, sl], fin[:])
                                else:
                                    nc.scalar.copy(o_sb[:, sl], fin[:])
                                if j == JB - 1:
                                    if h == 1 and cfg.get("tail_quarter", False):
                                        for qtr in range(2):
                                            q0 = h * 512 + qtr * 256
                                            nc.sync.dma_start(
                                                out[s0 + j * P : s0 + (j + 1) * P,
                                                    q0 : q0 + 256],
                                                o_sb[:, q0 : q0 + 256],
                                            )
                                    else:
                                        nc.sync.dma_start(
                                            out[s0 + j * P : s0 + (j + 1) * P, sl],
                                            o_sb[:, sl],
                                        )
                            if j < JB - 1:
                                nc.sync.dma_start(
                                    out[s0 + j * P : s0 + (j + 1) * P, :],
                                    o_sb[:],
                                )
                        return
                    for j in range(JB):
                        o_sb = c_out.tile([P, E], f32, name="o_sb")
                        for h in range(2):
                            fin = c_fin_ps.tile([P, 512], f32, name="fin_ps")
                            for c in range(EC):
                                nc.tensor.matmul(
                                    fin[:],
                                    qn_tiles[c][:, j * P : (j + 1) * P],
                                    W2[c][:, h * 512 : (h + 1) * 512],
                                    start=(c == 0),
                                    stop=(c == EC - 1),
                                )
                            sl = slice(h * 512, (h + 1) * 512)
                            if with_bias:
                                nc.vector.scalar_tensor_tensor(
                                    o_sb[:, sl], fin[:], 0.0,
                                    bo_bcast[:, h * 512 : (h + 1) * 512],
                                    ALU.add, ALU.add,
                                )
                            else:
                                nc.vector.tensor_copy(o_sb[:, sl], fin[:])
                        nc.sync.dma_start(
                            out[s0 + j * P : s0 + (j + 1) * P, :], o_sb[:]
                        )

                # pipeline: projA(0) | B | normB(0) | projA(1) consume(0)
                # normB(1) | projA(2) consume(1) normB(2) | ... | consume(7)
                qt0, qss0_ps, qss0_mm = projA(0, xq0_blk, qss_late=True)

                # ---- PHASE B: W2 (psum borrowed from c_fin pool) ----
                def phaseB(cs):
                    for c in cs:
                        for h in range(2):
                            if (2 * c + h) % 2 == 1 and cfg.get("b_alt", False):
                                w2p = c_pj_ps.tile([P, 512], f32, name="q_pj")
                            else:
                                w2p = c_fin_ps.tile([P, 512], f32, name="fin_ps")
                            nc.tensor.matmul(
                                w2p[:],
                                bd[c][:],
                                WoT[c][:, h * 512 : (h + 1) * 512],
                                start=True,
                                stop=True,
                            )
                            if h == 0:
                                nc.vector.tensor_scalar(
                                    W2[c][:, h * 512 : (h + 1) * 512],
                                    w2p[:],
                                    invk[:, c : c + 1],
                                    None,
                                    ALU.mult,
                                )
                            else:
                                nc.scalar.mul(
                                    W2[c][:, h * 512 : (h + 1) * 512],
                                    w2p[:],
                                    invk[:, c : c + 1],
                                )

                if cfg.get("pool_norm", False):
                    qn_prev = projA_pool(0, xq0_blk)
                    phaseB(range(EC))
                    for blk_i in range(1, NBLK):
                        qn_cur = projA_pool(blk_i)
                        consume(blk_i - 1, qn_prev)
                        qn_prev = qn_cur
                    consume(NBLK - 1, qn_prev)
                    _patch_bass(nc)
                    return nc

                nb_split = cfg.get("split_b", True)
                phaseB(range(4) if nb_split else range(EC))
                if taps:
                    nc.sync.dma_start(taps["tap_w20"], W2[0][:])
                qss0_mm()
                qn_prev = normB(_finish_ss(qt0, qss0_ps),
                                on_pe=cfg.get("qb_pe_all", False))
                for blk_i in range(1, NBLK):
                    st_cur = projA(blk_i)
                    if blk_i == 1 and nb_split:
                        phaseB(range(4, EC))
                    if blk_i == NBLK - 1:
                        if cfg.get("tail_consume_first", False):
                            consume(blk_i - 1, qn_prev)
                            qn_prev = normB(st_cur)
                        else:
                            qn_cur = normB(
                                st_cur,
                                on_pe=not cfg.get("last_dma_qb", True),
                                split4=cfg.get("tail_split4", 2),
                            )
                            consume(blk_i - 1, qn_prev)
                            qn_prev = qn_cur
                    else:
                        qn_cur = normB(st_cur, on_pe=cfg.get("qb_pe_all", False))
                        consume(blk_i - 1, qn_prev)
                        qn_prev = qn_cur
                consume(NBLK - 1, qn_prev)

    _patch_bass(nc)
    return nc


# --------------------------------------------------------------------------
# Host wrapper
# --------------------------------------------------------------------------
_NC_CACHE = {}


def _get_nc(S, with_bias=True):
    key = (S, with_bias)
    if key not in _NC_CACHE:
        _NC_CACHE[key] = build(S, with_bias)
    return _NC_CACHE[key]


def make_in_maps(query, key, value, Wq, bq, Wk, bk, Wv, bv, Wo, bo):
    import ml_dtypes

    bf = ml_dtypes.bfloat16
    query = np.asarray(query, np.float32).astype(bf)
    key = np.asarray(key, np.float32).astype(bf)
    value = np.asarray(value, np.float32).astype(bf)
    B = query.shape[0]
    bones = np.zeros((P, EC * H), np.float32)
    bpick = np.zeros((H, EC * P), np.float32)
    for c in range(EC):
        bones[0:D, c * H + 2 * c] = 1.0
        bones[D:P, c * H + 2 * c + 1] = 1.0
        bpick[2 * c, c * P : c * P + D] = 1.0
        bpick[2 * c + 1, c * P + D : (c + 1) * P] = 1.0
    shared = {
        "bones": np.ascontiguousarray(bones.astype(bf)),
        "bpick": np.ascontiguousarray(bpick.astype(bf)),
        "WqT": np.ascontiguousarray(np.asarray(Wq, np.float32).T.astype(bf)),
        "WkT": np.ascontiguousarray(np.asarray(Wk, np.float32).T.astype(bf)),
        "WvT": np.ascontiguousarray(np.asarray(Wv, np.float32).T.astype(bf)),
        "WoT": np.ascontiguousarray(np.asarray(Wo, np.float32).T.astype(bf)),
        "bq": np.ascontiguousarray(np.asarray(bq, np.float32).reshape(1, E)),
        "bk": np.ascontiguousarray(np.asarray(bk, np.float32).reshape(1, E)),
        "bv": np.ascontiguousarray(np.asarray(bv, np.float32).reshape(1, E)),
        "bo": np.ascontiguousarray(np.asarray(bo, np.float32).reshape(1, E)),
    }
    return [
        {
            "xqT": np.ascontiguousarray(query[c].T),
            "xkT": np.ascontiguousarray(key[c].T),
            "xvT": np.ascontiguousarray(value[c].T),
            **shared,
        }
        for c in range(B)
    ]


def kernel(query, key, value, Wq, bq, Wk, bk, Wv, bv, Wo, bo):
    query = np.asarray(query, np.float32)
    B, S, E_ = query.shape
    assert E_ == E and B == N_CORES
    in_maps = make_in_maps(query, key, value, Wq, bq, Wk, bk, Wv, bv, Wo, bo)
    with_bias = any(
        np.any(np.asarray(b)) for b in (bq, bk, bv, bo)
    )
    nc = _get_nc(S, with_bias)
    res = run_bass_kernel_spmd(nc, in_maps, core_ids=list(range(N_CORES)))
    return np.stack([res.results[c]["out"] for c in range(B)])

